# revision 21
# baseline (speedup 1.0000x reference)
"""Trainium2 Bass kernel for AnyModalMirasol (local+global block-causal transformer).

Sharding: data-parallel over the 16 (batch, group) sequences, zig-zag assigned:
core c owns (b=0, g=c) and (b=1, g=7-c).  Local attention is core-local.
Global attention is HEAD-sharded: core c owns heads 2c, 2c+1.  Three AllToAll
exchanges per block: (1) normed q/k feature-slices, (2) v head-slices
(token-major, with a per-head ones column so AV also yields the softmax
denominator), (3) attention outputs back to token owners.  Causality is exact:
key blocks stream only causally-visible query ranges; only diagonal 128x128
corners need a static triangular mask.  Global mem-token queries (attend only
to mem keys) are computed redundantly on every core for all heads.

Compute: f32 PSUM accumulation, bf16 operands on the PE.
"""

import os
import sys

sys.path.insert(0, "/opt/trn_rl_repo")
os.environ.setdefault("MYCRO_LOCAL_CACHE", "1")

import numpy as np
from contextlib import ExitStack

import concourse.bass as bass
import concourse.mybir as mybir
import concourse.tile as tile
from concourse.bass_utils import run_bass_kernel_spmd

F32 = mybir.dt.float32
BF = mybir.dt.bfloat16
AF = mybir.ActivationFunctionType
ALU = mybir.AluOpType

C = 1024
KT = 8          # C / 128
NH = 16
HD = 64
NB = 2
T = 256
ML = 16         # local mem tokens per group
MG = 16         # global mem tokens per batch
NCORE = 8
TL = 272        # ML + T tokens per local sequence; also MG + T per global group
S2 = 2 * TL     # both sequences side by side
NQ = 2048       # global x tokens per batch
VW = 65         # per-head v width (64 + ones column)
EPS = 1e-6


def build_nc():
    nc = bass.Bass()

    p = {}
    p["xx"] = nc.declare_dram_parameter("xx", [2, T, C], F32, isOutput=False)
    p["lm"] = nc.declare_dram_parameter("lm", [2, ML, C], F32, isOutput=False)
    p["mem"] = nc.declare_dram_parameter("mem", [2, MG, C], F32, isOutput=False)
    p["wqk_loc"] = nc.declare_dram_parameter("wqk_loc", [NB, 16, 128, KT * 128], BF, isOutput=False)
    p["wv_loc"] = nc.declare_dram_parameter("wv_loc", [NB, 2, 128, KT * 512], BF, isOutput=False)
    p["wp_loc"] = nc.declare_dram_parameter("wp_loc", [NB, 8, 128, KT * 128], BF, isOutput=False)
    p["wqk_glob"] = nc.declare_dram_parameter("wqk_glob", [NB, 16, 128, KT * 128], BF, isOutput=False)
    p["wv_glob"] = nc.declare_dram_parameter("wv_glob", [NB, 2, 128, KT * 512], BF, isOutput=False)
    p["wp_glob"] = nc.declare_dram_parameter("wp_glob", [NB, 8, 128, KT * 128], BF, isOutput=False)
    p["gT_loc"] = nc.declare_dram_parameter("gT_loc", [NB, 128, KT], F32, isOutput=False)
    p["gT_glob"] = nc.declare_dram_parameter("gT_glob", [NB, 128, KT], F32, isOutput=False)
    p["corner"] = nc.declare_dram_parameter("corner", [128, 128], BF, isOutput=False)
    p["trimem"] = nc.declare_dram_parameter("trimem", [ML, TL], BF, isOutput=False)
    p["mask_memq"] = nc.declare_dram_parameter("mask_memq", [MG, MG], BF, isOutput=False)
    p["onesc"] = nc.declare_dram_parameter("onesc", [128, 1], BF, isOutput=False)
    p["onesr"] = nc.declare_dram_parameter("onesr", [1, 128], BF, isOutput=False)
    p["selp"] = nc.declare_dram_parameter("selp", [97, 4 * 64], BF, isOutput=False)
    p["id128"] = nc.declare_dram_parameter("id128", [128, 128], F32, isOutput=False)
    p["out"] = nc.declare_dram_parameter("out", [2, T, C], F32, isOutput=True)

    with nc.allow_low_precision(reason="bf16 compute, f32 accumulation in PSUM"):
        with tile.TileContext(nc) as tc:
            with ExitStack() as ctx:
                build_body(ctx, tc, nc, p)
    split_excess_waits(nc)
    return nc


MAX_WAITS = 1      # this walrus build: 1 sync wait per instruction


def split_excess_waits(nc):
    """Hoist overflow semaphore waits onto preceding same-engine nops."""
    nsplit = 0
    for f in nc.m.functions:
        for blk in f.blocks:
            il = blk.instructions
            i = 0
            while i < len(il):
                inst = il[i]
                si = getattr(inst, "sync_info", None)
                if si is not None and si.on_wait and len(si.on_wait) > MAX_WAITS:
                    waits = list(si.on_wait)
                    keep = waits[-MAX_WAITS:]
                    excess = waits[:-MAX_WAITS]
                    pos = i
                    while excess:
                        chunk, excess = excess[:MAX_WAITS], excess[MAX_WAITS:]
                        nop = mybir.InstNoOp(name=f"{inst.name}_wsplit{nsplit}", ins=[], outs=[])
                        nsplit += 1
                        nop.engine = inst.engine
                        nop.sync_info = mybir.SyncInfo(on_wait=chunk, on_update=[])
                        nc.register_instruction(nop, overwrite=True)
                        il.insert(pos, nop)
                        pos += 1
                        i += 1
                    inst.sync_info = mybir.SyncInfo(on_wait=keep, on_update=list(si.on_update))
                i += 1
    return nsplit


def build_body(ctx, tc, nc, p):
    const = ctx.enter_context(tc.tile_pool(name="const", bufs=1))
    state = ctx.enter_context(tc.tile_pool(name="state", bufs=1))
    wpool = ctx.enter_context(tc.tile_pool(name="wpool", bufs=1))
    spool = ctx.enter_context(tc.tile_pool(name="spool", bufs=1))
    psum = ctx.enter_context(tc.tile_pool(name="psum", bufs=1, space="PSUM"))
    dram = ctx.enter_context(tc.tile_pool(name="dram", bufs=1, space="DRAM"))

    def pt(shape, name, tag="ps", bufs=4):
        return psum.tile(shape, F32, tag=tag, name=name, bufs=bufs)

    # ---------------- constants ----------------
    corner = const.tile([128, 128], BF, name="corner_sb")
    nc.sync.dma_start(out=corner, in_=p["corner"][:, :])
    trimem = const.tile([ML, TL], BF, name="trimem_sb")
    nc.sync.dma_start(out=trimem, in_=p["trimem"][:, :])
    mask_memq = const.tile([MG, MG], BF, name="mask_memq_sb")
    nc.sync.dma_start(out=mask_memq, in_=p["mask_memq"][:, :])
    onesc = const.tile([128, 1], BF, name="onesc_sb")
    nc.sync.dma_start(out=onesc, in_=p["onesc"][:, :])
    onesr = const.tile([1, 128], BF, name="onesr_sb")
    nc.sync.dma_start(out=onesr, in_=p["onesr"][:, :])
    selp = const.tile([97, 4 * 64], BF, name="selp_sb")
    nc.sync.dma_start(out=selp, in_=p["selp"][:, :])
    id128 = const.tile([128, 128], F32, name="id128_sb")
    nc.sync.dma_start(out=id128, in_=p["id128"][:, :])
    id128b = const.tile([128, 128], BF, name="id128b_sb")
    nc.vector.tensor_copy(id128b, id128)
    zb = const.tile([128, 1], F32, name="zb_sb")
    nc.vector.memset(zb, 0.0)
    zrow = const.tile([1, 128], BF, name="zrow_sb")
    nc.vector.memset(zrow, 0.0)
    ones512 = const.tile([1, 512], BF, name="ones512_sb")
    nc.vector.memset(ones512, 1.0)
    epsb = const.tile([1, 1], F32, name="epsb_sb")
    nc.vector.memset(epsb, EPS)
    gT = {}
    for wt in ("loc", "glob"):
        for i in range(NB):
            g = const.tile([128, KT], F32, name=f"gT_{wt}{i}_sb")
            nc.sync.dma_start(out=g, in_=p[f"gT_{wt}"][i])
            gT[(wt, i)] = g

    # ---------------- persistent state ----------------
    xt = [[state.tile([128, TL], BF, name=f"xt_{s}_{kt}") for kt in range(KT)] for s in range(2)]
    mt = [state.tile([128, 2 * MG], BF, name=f"mt_{kt}") for kt in range(KT)]

    # q/k/y tiles: [128, 544] with both sequences side by side (cols s*TL..)
    qT = [state.tile([128, S2], BF, name=f"qT_{kt}") for kt in range(KT)]
    kT_ = [state.tile([128, S2], BF, name=f"kT_{kt}") for kt in range(KT)]
    yT = [state.tile([128, S2], BF, name=f"yT_{kt}") for kt in range(KT)]
    qTg = [state.tile([128, S2], BF, name=f"qTg_{kt}") for kt in range(KT)]
    kTg = [state.tile([128, S2], BF, name=f"kTg_{kt}") for kt in range(KT)]
    yTg = [state.tile([128, S2], BF, name=f"yTg_{kt}") for kt in range(KT)]
    xg = [[state.tile([128, TL], BF, name=f"xg_{g}_{kt}") for kt in range(KT)] for g in range(2)]

    # v tiles (token-major, NH heads x VW cols, ones column preset)
    def make_v(nm):
        vm = [state.tile([ML, NH * VW], BF, name=f"vm{nm}_{s}") for s in range(2)]
        vx = [[state.tile([128, NH * VW], BF, name=f"vx{nm}_{s}_{j}") for j in range(2)] for s in range(2)]
        for s in range(2):
            nc.vector.memset(vm[s].rearrange("p (h w) -> p h w", h=NH)[:, :, 64:65], 1.0)
            for j in range(2):
                nc.vector.memset(vx[s][j].rearrange("p (h w) -> p h w", h=NH)[:, :, 64:65], 1.0)
        return vm, vx

    vml, vxl = make_v("l")
    vmg, vxg = make_v("g")

    # gathered / exchanged global-attention tiles
    k_all = state.tile([128, 8 * S2], BF, name="k_all")       # per src: [128, 544]
    q_all = [state.tile([128, NQ], BF, name=f"q_all_{b}") for b in range(2)]
    y_all = [state.tile([128, NQ], BF, name=f"y_all_{b}") for b in range(2)]
    vb = [state.tile([128, 8 * 2 * 130], BF, name=f"vb_{b}") for b in range(2)]
    vg_m = [state.tile([MG, 130], BF, name=f"vg_m_{b}") for b in range(2)]

    # ---------------- initial load + transpose ----------------
    with nc.named_scope("init"):
        for s in range(2):
            xtm = [spool.tile([128, C], F32, name=f"xtm_{s}_{rb}", tag=f"xtm_{rb}") for rb in range(2)]
            for rb in range(2):
                nc.sync.dma_start(out=xtm[rb], in_=p["xx"][s, rb * 128:(rb + 1) * 128, :])
            lmm = spool.tile([ML, C], F32, name=f"lmm_{s}", tag="lmm")
            nc.sync.dma_start(out=lmm, in_=p["lm"][s])
            for kt in range(KT):
                for rb in range(2):
                    ps_t = pt([128, 128], f"ps_tr_{s}_{kt}_{rb}")
                    nc.tensor.transpose(ps_t, xtm[rb][:, kt * 128:(kt + 1) * 128], id128)
                    nc.vector.tensor_copy(xt[s][kt][:, ML + rb * 128: ML + (rb + 1) * 128], ps_t)
                ps_t = pt([128, ML], f"ps_trl_{s}_{kt}")
                nc.tensor.transpose(ps_t, lmm[:, kt * 128:(kt + 1) * 128], id128[:ML, :ML])
                nc.vector.tensor_copy(xt[s][kt][:, 0:ML], ps_t)
        for b in range(2):
            memm = spool.tile([MG, C], F32, name=f"memm_{b}", tag="lmm")
            nc.sync.dma_start(out=memm, in_=p["mem"][b])
            for kt in range(KT):
                ps_t = pt([128, MG], f"ps_trm_{b}_{kt}")
                nc.tensor.transpose(ps_t, memm[:, kt * 128:(kt + 1) * 128], id128[:MG, :MG])
                nc.vector.tensor_copy(mt[kt][:, b * MG:(b + 1) * MG], ps_t)

    # ---------------- stage building blocks ----------------
    def qk_norm(i, wname, gv, xs, qdst, kdst, stage):
        """q^T/k^T feature-major into [128, 544] tiles, rms-normed * g."""
        for part, dst in (("q", qdst), ("k", kdst)):
            moff = 0 if part == "q" else 8
            sumsq = [None, None]
            for mtile in range(KT):
                w = wpool.tile([128, KT * 128], BF, tag="wqk", bufs=3, name=f"w_{stage}{i}{part}_{mtile}")
                nc.sync.dma_start(out=w, in_=p[wname][i, moff + mtile])
                ps2 = [pt([128, TL], f"ps_{stage}{i}{part}_{mtile}_{s}") for s in range(2)]
                for kt in range(KT):
                    for s in range(2):
                        nc.tensor.matmul(ps2[s], w[:, kt * 128:(kt + 1) * 128], xs[s][kt],
                                         start=(kt == 0), stop=(kt == KT - 1))
                for s in range(2):
                    dsl = dst[mtile][:, s * TL:(s + 1) * TL]
                    nc.vector.tensor_copy(dsl, ps2[s])
                    sq = spool.tile([128, TL], BF, tag="sq", bufs=3, name=f"sq_{stage}{i}{part}_{mtile}_{s}")
                    nc.vector.tensor_mul(sq, dsl, dsl)
                    if sumsq[s] is None:
                        sumsq[s] = pt([1, TL], f"ss_{stage}{i}{part}_{s}", tag="ssacc", bufs=2)
                    nc.tensor.matmul(sumsq[s], onesc, sq, start=(mtile == 0), stop=(mtile == KT - 1))
            for s in range(2):
                rs = spool.tile([1, TL], F32, tag="rs", bufs=2, name=f"rs_{stage}{i}{part}_{s}")
                nc.scalar.activation(rs, sumsq[s], AF.Sqrt, scale=1.0 / C, bias=epsb)
                rrb = spool.tile([1, TL], BF, tag="rrb", bufs=2, name=f"rrb_{stage}{i}{part}_{s}")
                nc.vector.reciprocal(rrb, rs)
                ps_bc = pt([128, TL], f"bc_{stage}{i}{part}_{s}")
                nc.tensor.matmul(ps_bc, onesr, rrb, start=True, stop=True)
                bc = spool.tile([128, TL], BF, tag="bc", bufs=2, name=f"bcs_{stage}{i}{part}_{s}")
                nc.scalar.activation(bc, ps_bc, AF.Copy)
                for mtile in range(KT):
                    dsl = dst[mtile][:, s * TL:(s + 1) * TL]
                    nc.vector.tensor_mul(dsl, dsl, bc)
                    nc.vector.tensor_scalar_mul(dsl, dsl, gv[:, mtile:mtile + 1])

    def v_compute(i, wname, xs, vm, vx, stage):
        """v token-major with per-head ones columns (preset)."""
        for nch in range(2):
            w = wpool.tile([128, KT * 512], BF, tag="wv", bufs=2, name=f"wv_{stage}{i}_{nch}")
            nc.sync.dma_start(out=w, in_=p[wname][i, nch])
            blocks = [(s, vm[s], 0, ML) for s in range(2)]
            blocks += [(s, vx[s][j], ML + j * 128, 128) for s in range(2) for j in range(2)]
            for s, dst, ts_, rows in blocks:
                ps = pt([128, 512], f"psv_{stage}{i}_{nch}_{s}_{ts_}")
                for kt in range(KT):
                    nc.tensor.matmul(ps[:rows], xs[s][kt][:, ts_:ts_ + rows], w[:, kt * 512:(kt + 1) * 512],
                                     start=(kt == 0), stop=(kt == KT - 1))
                dstv = dst[:rows, nch * 8 * VW:(nch + 1) * 8 * VW].rearrange("p (h w) -> p h w", h=8)
                nc.vector.tensor_copy(dstv[:, :, 0:64], ps[:rows].rearrange("p (h w) -> p h w", h=8))

    def y_raw_write(tag, ps_y, n, wdst, dcol, drow):
        """wdst [64, n] = raw ps_y[0:64, :n]; denominator row -> dcol at partition 32*drow."""
        nc.vector.tensor_copy(wdst, ps_y[0:64, :n])
        nc.scalar.activation(dcol[32 * drow:32 * drow + 1, :n], ps_y[64:65, :n], AF.Copy)

    def y_norm_finish(tag, dcol, n, items):
        """items: up to 4 (raw-y dest, base-partition) pairs; denominators at
        partitions 0/32/64/96 of dcol [97, n].  One reciprocal; per dest:
        selector-broadcast into the dest's partition range + in-place mul."""
        rrb = spool.tile([97, 512], BF, tag="rrbA", bufs=2, name=f"rrbA_{tag}")
        nc.vector.reciprocal(rrb[:, :n], dcol[:, :n])
        for r, (wdst, off) in enumerate(items):
            ps_bc = pt([128, 512], f"bca_{tag}_{r}")
            nc.tensor.matmul(ps_bc[off:off + 64, :n], selp[:, r * 64:(r + 1) * 64], rrb[:, :n],
                             start=True, stop=True)
            bc = spool.tile([128, 512], BF, tag="bcA", bufs=4, name=f"bcA_{tag}_{r}")
            nc.vector.tensor_copy(bc[off:off + 64, :n], ps_bc[off:off + 64, :n])
            nc.vector.tensor_mul(wdst, wdst, bc[off:off + 64, :n])

    def local_attn(i):
        pend = []
        for s in range(2):
            dloc = None
            wdsts = []
            for t in range(KT):
                for hh in range(2):
                    off = hh * 64
                    tag = f"L{i}_{s}_{t}_{hh}"
                    h = 2 * t + hh
                    q = qT[t][off:off + 64, s * TL:(s + 1) * TL]
                    ps_y = pt([VW, TL], f"ya_{tag}", tag="psy", bufs=2)
                    # mem keys (16): visible to everything; triangular vs mem queries
                    ps_s = pt([128, 512], f"sa_{tag}_m")
                    nc.tensor.matmul(ps_s[:ML, :TL], kT_[t][off:off + 64, s * TL:s * TL + ML], q,
                                     start=True, stop=True)
                    es = spool.tile([128, 512], BF, tag="es", bufs=6, name=f"es_{tag}_m")
                    nc.scalar.activation(es[:ML, :TL], ps_s[:ML, :TL], AF.Exp, scale=0.125, bias=zb[:ML])
                    nc.vector.tensor_mul(es[:ML, :TL], es[:ML, :TL], trimem)
                    nc.tensor.matmul(ps_y, vml[s][0:ML, h * VW:(h + 1) * VW], es[:ML, :TL],
                                     start=True, stop=False)
                    # x key block 0 (keys 16:144): queries 16:272
                    ps_s = pt([128, 512], f"sa_{tag}_0")
                    nc.tensor.matmul(ps_s[:, :T], kT_[t][off:off + 64, s * TL + ML:s * TL + ML + 128],
                                     q[:, ML:TL], start=True, stop=True)
                    es = spool.tile([128, 512], BF, tag="es", bufs=6, name=f"es_{tag}_0")
                    nc.scalar.activation(es[:, :T], ps_s[:, :T], AF.Exp, scale=0.125, bias=zb)
                    nc.vector.tensor_mul(es[:, 0:128], es[:, 0:128], corner)
                    nc.tensor.matmul(ps_y[:, ML:TL], vxl[s][0][:, h * VW:(h + 1) * VW], es[:, :T],
                                     start=False, stop=False)
                    # x key block 1 (keys 144:272): queries 144:272
                    ps_s = pt([128, 512], f"sa_{tag}_1")
                    nc.tensor.matmul(ps_s[:, :128], kT_[t][off:off + 64, s * TL + ML + 128:(s + 1) * TL],
                                     q[:, ML + 128:TL], start=True, stop=True)
                    es = spool.tile([128, 512], BF, tag="es", bufs=6, name=f"es_{tag}_1")
                    nc.scalar.activation(es[:, :128], ps_s[:, :128], AF.Exp, scale=0.125, bias=zb)
                    nc.vector.tensor_mul(es[:, :128], es[:, :128], corner)
                    nc.tensor.matmul(ps_y[:, ML + 128:TL], vxl[s][1][:, h * VW:(h + 1) * VW], es[:, :128],
                                     start=False, stop=True)
                    if dloc is None:
                        dloc = spool.tile([97, TL], F32, tag="dloc", bufs=8, name=f"dloc_{tag}")
                        nc.gpsimd.memset(dloc, 1.0)
                    wdst = yT[t][off:off + 64, s * TL:(s + 1) * TL]
                    wdsts.append((wdst, off))
                    y_raw_write(tag, ps_y, TL, wdst, dloc, len(wdsts) - 1)
                    if len(wdsts) == 4:
                        pend.append((f"L{i}_{s}_{t}{hh}", dloc, TL, wdsts))
                        dloc, wdsts = None, []

        for args in pend:
            y_norm_finish(*args)

    def memq(i):
        """global mem-token queries: keys = own-batch mem tokens, causal; all heads."""
        dmem = None
        wdsts = []
        pend = []
        for g in range(2):
            for t in range(KT):
                for hh in range(2):
                    off = hh * 64
                    tag = f"Gm{i}_{g}_{t}_{hh}"
                    h = 2 * t + hh
                    ps_s = pt([MG, MG], f"ms_{tag}", tag="ps", bufs=4)
                    nc.tensor.matmul(ps_s, kTg[t][off:off + 64, g * TL:g * TL + MG],
                                     qTg[t][off:off + 64, g * TL:g * TL + MG], start=True, stop=True)
                    es = spool.tile([MG, MG], BF, tag="esm", bufs=2, name=f"esm_{tag}")
                    nc.scalar.activation(es, ps_s, AF.Exp, scale=0.125, bias=zb[:MG])
                    nc.vector.tensor_mul(es, es, mask_memq)
                    ps_y = pt([VW, MG], f"ym_{tag}", tag="psy", bufs=2)
                    nc.tensor.matmul(ps_y, vmg[g][0:MG, h * VW:(h + 1) * VW], es, start=True, stop=True)
                    if dmem is None:
                        dmem = spool.tile([97, MG], F32, tag="dmem", bufs=8, name=f"dmem_{tag}")
                        nc.gpsimd.memset(dmem, 1.0)
                    wdst = yTg[t][off:off + 64, g * TL:g * TL + MG]
                    wdsts.append((wdst, off))
                    y_raw_write(tag, ps_y, MG, wdst, dmem, len(wdsts) - 1)
                    if len(wdsts) == 4:
                        pend.append((f"Gm{i}_{g}_{t}{hh}", dmem, MG, wdsts))
                        dmem, wdsts = None, []

        for args in pend:
            y_norm_finish(*args)

    def global_attn(i):
        """head-sharded causal attention over gathered q/k/v; writes y_all."""
        pend = []
        for b in range(2):
            dglb = None
            wdsts = []
            for lh in range(2):
                off = lh * 64
                for qc in range(4):
                    tag = f"G{i}_{b}_{lh}_{qc}"
                    qsl = q_all[b][off:off + 64, qc * 512:(qc + 1) * 512]
                    ps_y = pt([VW, 512], f"ya_{tag}", tag="psy", bufs=2)
                    # global mem keys: visible to all x queries (from src 0's copy)
                    ps_s = pt([128, 512], f"sa_{tag}_m")
                    nc.tensor.matmul(ps_s[:MG, :], k_all[off:off + 64, b * TL:b * TL + MG], qsl,
                                     start=True, stop=True)
                    es = spool.tile([128, 512], BF, tag="es", bufs=6, name=f"es_{tag}_m")
                    nc.scalar.activation(es[:MG, :], ps_s[:MG, :], AF.Exp, scale=0.125, bias=zb[:MG])
                    nc.tensor.matmul(ps_y, vg_m[b][0:MG, lh * VW:(lh + 1) * VW], es[:MG, :],
                                     start=True, stop=False)
                    # full + band x key blocks
                    for m in range(4 * qc + 4):
                        jb = (m >> 1) if b == 0 else 7 - (m >> 1)
                        r = m - 4 * qc  # >= 0 for band blocks
                        ks = k_all[off:off + 64, jb * S2 + b * TL + ML + (m & 1) * 128:
                                   jb * S2 + b * TL + ML + (m & 1) * 128 + 128]
                        vs = vb[b][:, (2 * jb + (m & 1)) * 130 + lh * VW:
                                   (2 * jb + (m & 1)) * 130 + (lh + 1) * VW]
                        ps_s = pt([128, 512], f"sa_{tag}_{m}")
                        es = spool.tile([128, 512], BF, tag="es", bufs=6, name=f"es_{tag}_{m}")
                        if r < 0:
                            nc.tensor.matmul(ps_s, ks, qsl, start=True, stop=True)
                            nc.scalar.activation(es, ps_s, AF.Exp, scale=0.125, bias=zb)
                            nc.tensor.matmul(ps_y, vs, es, start=False, stop=False)
                        else:
                            n = 512 - r * 128
                            nc.tensor.matmul(ps_s[:, :n], ks, qsl[:, r * 128:], start=True, stop=True)
                            nc.scalar.activation(es[:, :n], ps_s[:, :n], AF.Exp, scale=0.125, bias=zb)
                            nc.vector.tensor_mul(es[:, :128], es[:, :128], corner)
                            nc.tensor.matmul(ps_y[:, r * 128:], vs, es[:, :n],
                                             start=False, stop=(m == 4 * qc + 3))
                    if dglb is None:
                        dglb = spool.tile([97, 512], F32, tag="dglb", bufs=4, name=f"dglb_{tag}")
                        nc.gpsimd.memset(dglb, 1.0)
                    wdst = y_all[b][off:off + 64, qc * 512:(qc + 1) * 512]
                    wdsts.append((wdst, off))
                    y_raw_write(tag, ps_y, 512, wdst, dglb, len(wdsts) - 1)
                    if len(wdsts) == 4:
                        pend.append((f"G{i}_{b}_{lh}{qc}", dglb, 512, wdsts))
                        dglb, wdsts = None, []
        return pend

    def proj(i, wname, ysrc, wr, stage):
        for mo in range(KT):
            w = wpool.tile([128, KT * 128], BF, tag="wp", bufs=2, name=f"wp_{stage}{i}_{mo}")
            nc.sync.dma_start(out=w, in_=p[wname][i, mo])
            ps2 = [pt([128, TL], f"psp_{stage}{i}_{mo}_{s}") for s in range(2)]
            for kt in range(KT):
                for s in range(2):
                    nc.tensor.matmul(ps2[s], w[:, kt * 128:(kt + 1) * 128], ysrc[kt][:, s * TL:(s + 1) * TL],
                                     start=(kt == 0), stop=(kt == KT - 1))
            for s in range(2):
                wr(s, mo, ps2[s])

    # ================= main blocks =================
    for i in range(NB):
        # ---------- local stage ----------
        with nc.named_scope(f"Lqkv{i}"):
            v_compute(i, "wv_loc", xt, vml, vxl, "L")
            qk_norm(i, "wqk_loc", gT[("loc", i)], xt, qT, kT_, "L")
        with nc.named_scope(f"Lattn{i}"):
            local_attn(i)

        def wr_loc(s, mo, ps, i=i):
            nc.scalar.activation(xt[s][mo], ps, AF.Copy)

        with nc.named_scope(f"Lproj{i}"):
            proj(i, "wp_loc", yT, wr_loc, "L")

        # ---------- global stage ----------
        with nc.named_scope(f"Gqkv{i}"):
            for g in range(2):
                for kt in range(KT):
                    nc.gpsimd.tensor_copy(xg[g][kt][:, 0:MG], mt[kt][:, g * MG:(g + 1) * MG])
                    nc.gpsimd.tensor_copy(xg[g][kt][:, MG:TL], xt[g][kt][:, ML:TL])
            qk_norm(i, "wqk_glob", gT[("glob", i)], xg, qTg, kTg, "G")

            # AllToAll 1: q/k head-slices.  chunk d = [q rows 0:128 ; k rows 128:256]
            a2a_qk_in = dram.tile([8 * 256, S2], BF, name=f"a2aqk_in_{i}")
            a2a_qk_out = dram.tile([8 * 256, S2], BF, name=f"a2aqk_out_{i}")
            for d in range(8):
                nc.sync.dma_start(out=a2a_qk_in[d * 256:d * 256 + 128, :], in_=qTg[d])
                nc.sync.dma_start(out=a2a_qk_in[d * 256 + 128:(d + 1) * 256, :], in_=kTg[d])
            rg = [list(range(NCORE))]
            nc.gpsimd.collective_compute("AllToAll", ALU.bypass, replica_groups=rg,
                                         ins=[a2a_qk_in.opt()], outs=[a2a_qk_out.opt()])

            v_compute(i, "wv_glob", xg, vmg, vxg, "G")

            # AllToAll 2: v head-slices, token-major [544 rows/src, 130]
            a2a_v_in = dram.tile([8 * S2, 130], BF, name=f"a2av_in_{i}")
            a2a_v_out = dram.tile([8 * S2, 130], BF, name=f"a2av_out_{i}")
            vin_p = a2a_v_in.rearrange("(d r) w -> r d w", d=8)
            for s in range(2):
                nc.sync.dma_start(out=vin_p[s * TL:s * TL + MG],
                                    in_=vmg[s].rearrange("p (d w) -> p d w", d=8))
                for j in range(2):
                    nc.sync.dma_start(out=vin_p[s * TL + MG + j * 128:s * TL + MG + (j + 1) * 128],
                                        in_=vxg[s][j].rearrange("p (d w) -> p d w", d=8))
            nc.gpsimd.collective_compute("AllToAll", ALU.bypass, replica_groups=rg,
                                         ins=[a2a_v_in.opt()], outs=[a2a_v_out.opt()])

        # overlap the collectives with mem-query attention
        with nc.named_scope(f"Gmemq{i}"):
            memq(i)

        with nc.named_scope(f"Gattn{i}"):
            # load exchanged tensors
            qko_p = a2a_qk_out.rearrange("(j r) w -> r j w", j=8)
            nc.gpsimd.dma_start(out=k_all.rearrange("p (j w) -> p j w", j=8), in_=qko_p[128:256])
            nc.gpsimd.dma_start(out=q_all[0].rearrange("p (j w) -> p j w", j=8),
                                in_=qko_p[0:128, :, MG:TL])
            for j in range(8):
                nc.gpsimd.dma_start(out=q_all[1][:, (7 - j) * 256:(8 - j) * 256],
                                    in_=a2a_qk_out[j * 256:j * 256 + 128, TL + MG:S2])
            vo_p = a2a_v_out.rearrange("(j r) w -> r j w", j=8)
            for b in range(2):
                for u in range(2):
                    nc.gpsimd.dma_start(
                        out=vb[b].rearrange("p (j u w) -> p j u w", j=8, u=2)[:, :, u],
                        in_=vo_p[b * TL + MG + u * 128:b * TL + MG + (u + 1) * 128])
                nc.gpsimd.dma_start(out=vg_m[b], in_=a2a_v_out[b * TL:b * TL + MG, :])
            for args in global_attn(i):
                y_norm_finish(*args)

            # AllToAll 3: y back to token owners.  chunk d rows 0:128 =
            # [cols 0:256 = b0 y of dest tokens, cols 256:512 = b1]
            a2a_y_in = dram.tile([8 * 128, 512], BF, name=f"a2ay_in_{i}")
            a2a_y_out = dram.tile([8 * 128, 512], BF, name=f"a2ay_out_{i}")
            yin_p = a2a_y_in.rearrange("(d r) w -> r d w", d=8)
            nc.sync.dma_start(out=yin_p[:, :, 0:256], in_=y_all[0].rearrange("p (d w) -> p d w", d=8))
            for d in range(8):
                nc.sync.dma_start(out=a2a_y_in[d * 128:(d + 1) * 128, 256:512],
                                    in_=y_all[1][:, (7 - d) * 256:(8 - d) * 256])
            nc.gpsimd.collective_compute("AllToAll", ALU.bypass, replica_groups=rg,
                                         ins=[a2a_y_in.opt()], outs=[a2a_y_out.opt()])
            for kt in range(KT):
                # yTg x cols (mem cols 0:16 written by memq)
                nc.gpsimd.dma_start(
                    out=yTg[kt].rearrange("p (s w) -> p s w", s=2)[:, :, MG:TL],
                    in_=a2a_y_out[kt * 128:(kt + 1) * 128, :].rearrange("r (s w) -> r s w", s=2))

        def wr_glob(s, mo, ps, i=i):
            nc.scalar.activation(xt[s][mo][:, ML:TL], ps[:, MG:TL], AF.Copy)
            nc.scalar.activation(mt[mo][:, s * MG:(s + 1) * MG], ps[:, 0:MG], AF.Copy)

        with nc.named_scope(f"Gproj{i}"):
            proj(i, "wp_glob", yTg, wr_glob, "G")

    # ================= output =================
    with nc.named_scope("out"):
        for s in range(2):
            osb = [spool.tile([128, C], F32, name=f"osb_{s}_{rb}", tag=f"xtm_{rb}") for rb in range(2)]
            for kt in range(KT):
                for rb in range(2):
                    ps_t = psum.tile([128, 128], BF, tag="ps", bufs=4, name=f"ps_out_{s}_{kt}_{rb}")
                    nc.tensor.transpose(ps_t, xt[s][kt][:, ML + rb * 128: ML + (rb + 1) * 128], id128b)
                    nc.vector.tensor_copy(osb[rb][:, kt * 128:(kt + 1) * 128], ps_t)
            for rb in range(2):
                nc.sync.dma_start(out=p["out"][s, rb * 128:(rb + 1) * 128, :], in_=osb[rb])


_NC_CACHE = None


def get_nc():
    global _NC_CACHE
    if _NC_CACHE is None:
        _NC_CACHE = build_nc()
    return _NC_CACHE


def make_in_maps(x, mem_tokens, local_mem, Wqkv_loc, Wproj_loc, g_loc, Wqkv_glob, Wproj_glob, g_glob):
    x = np.asarray(x, np.float32)
    mem_tokens = np.asarray(mem_tokens, np.float32)
    local_mem = np.asarray(local_mem, np.float32)
    g_loc = np.asarray(g_loc, np.float32)
    g_glob = np.asarray(g_glob, np.float32)

    import ml_dtypes
    bf16 = ml_dtypes.bfloat16

    def tile_qk(w):
        # (NB, C, 3C) -> qk part as [NB, 16, 128, KT*128]
        arr = np.asarray(w, np.float32).reshape(NB, KT, 128, 24, 128)
        qk = arr[:, :, :, 0:16, :].transpose(0, 3, 2, 1, 4).reshape(NB, 16, 128, KT * 128)
        return np.ascontiguousarray(qk).astype(bf16)

    def tile_v(w):
        arr = np.asarray(w, np.float32).reshape(NB, KT, 128, 24, 128)
        v = arr[:, :, :, 16:24, :].reshape(NB, KT, 128, 2, 4, 128)
        v = v.transpose(0, 3, 2, 1, 4, 5).reshape(NB, 2, 128, KT * 512)
        return np.ascontiguousarray(v).astype(bf16)

    def tile_p(w):
        arr = np.asarray(w, np.float32).reshape(NB, KT, 128, 8, 128)
        pr = arr.transpose(0, 3, 2, 1, 4).reshape(NB, 8, 128, KT * 128)
        return np.ascontiguousarray(pr).astype(bf16)

    gT_loc = np.ascontiguousarray(g_loc.reshape(NB, KT, 128).transpose(0, 2, 1))
    gT_glob = np.ascontiguousarray(g_glob.reshape(NB, KT, 128).transpose(0, 2, 1))

    kk = np.arange(128)
    corner = (kk[None, :] >= kk[:, None]).astype(np.float32)          # [k, q]: q >= k
    qq = np.arange(TL)
    km = np.arange(ML)
    trimem = (qq[None, :] >= km[:, None]).astype(np.float32)
    qm = np.arange(MG)
    mask_memq = (qm[None, :] >= qm[:, None]).astype(np.float32)
    onesc = np.ones((128, 1), np.float32)
    onesr = np.ones((1, 128), np.float32)
    selp = np.zeros((97, 4 * 64), np.float32)
    for r_ in range(4):
        selp[32 * r_, r_ * 64:(r_ + 1) * 64] = 1.0
    selp = selp.astype(bf16)
    id128 = np.eye(128, dtype=np.float32)

    base = dict(
        wqk_loc=tile_qk(Wqkv_loc), wv_loc=tile_v(Wqkv_loc), wp_loc=tile_p(Wproj_loc),
        wqk_glob=tile_qk(Wqkv_glob), wv_glob=tile_v(Wqkv_glob), wp_glob=tile_p(Wproj_glob),
        gT_loc=gT_loc, gT_glob=gT_glob,
        corner=corner.astype(bf16), trimem=trimem.astype(bf16), mask_memq=mask_memq.astype(bf16),
        onesc=onesc.astype(bf16), onesr=onesr.astype(bf16), id128=id128, selp=selp,
        mem=np.ascontiguousarray(mem_tokens),
    )

    in_maps = []
    for c in range(NCORE):
        m = dict(base)
        m["xx"] = np.ascontiguousarray(np.stack([x[0, c], x[1, 7 - c]]))
        m["lm"] = np.ascontiguousarray(np.stack([local_mem[0, c], local_mem[1, 7 - c]]))
        in_maps.append(m)
    return in_maps


def kernel(**inputs):
    nc = get_nc()
    in_maps = make_in_maps(**inputs)
    res = run_bass_kernel_spmd(nc, in_maps, list(range(NCORE)))
    out = np.zeros((2, NCORE, T, C), np.float32)
    for c in range(NCORE):
        o = res.results[c]["out"]
        out[0, c] = o[0]
        out[1, 7 - c] = o[1]
    return out


# revision 25
# speedup vs baseline: 1.0159x; 1.0159x over previous
"""Trainium2 Bass kernel for AnyModalMirasol (local+global block-causal transformer).

Sharding: data-parallel over the 16 (batch, group) sequences, zig-zag assigned:
core c owns (b=0, g=c) and (b=1, g=7-c).  Local attention is core-local.
Global attention is HEAD-sharded: core c owns heads 2c, 2c+1.  Three AllToAll
exchanges per block: (1) normed q/k feature-slices, (2) v head-slices
(token-major, with a per-head ones column so AV also yields the softmax
denominator), (3) attention outputs back to token owners.  Causality is exact:
key blocks stream only causally-visible query ranges; only diagonal 128x128
corners need a static triangular mask.  Global mem-token queries (attend only
to mem keys) are computed redundantly on every core for all heads.

Compute: f32 PSUM accumulation, bf16 operands on the PE.
"""

import os
import sys

sys.path.insert(0, "/opt/trn_rl_repo")
os.environ.setdefault("MYCRO_LOCAL_CACHE", "1")

import numpy as np
from contextlib import ExitStack

import concourse.bass as bass
import concourse.mybir as mybir
import concourse.tile as tile
from concourse.bass_utils import run_bass_kernel_spmd

F32 = mybir.dt.float32
BF = mybir.dt.bfloat16
AF = mybir.ActivationFunctionType
ALU = mybir.AluOpType

C = 1024
KT = 8          # C / 128
NH = 16
HD = 64
NB = 2
T = 256
ML = 16         # local mem tokens per group
MG = 16         # global mem tokens per batch
NCORE = 8
TL = 272        # ML + T tokens per local sequence; also MG + T per global group
S2 = 2 * TL     # both sequences side by side
NQ = 2048       # global x tokens per batch
VW = 65         # per-head v width (64 + ones column)
EPS = 1e-6


def build_nc():
    nc = bass.Bass()

    p = {}
    p["xx"] = nc.declare_dram_parameter("xx", [2, T, C], F32, isOutput=False)
    p["lm"] = nc.declare_dram_parameter("lm", [2, ML, C], F32, isOutput=False)
    p["mem"] = nc.declare_dram_parameter("mem", [2, MG, C], F32, isOutput=False)
    p["wqk_loc"] = nc.declare_dram_parameter("wqk_loc", [NB, 16, 128, KT * 128], BF, isOutput=False)
    p["wv_loc"] = nc.declare_dram_parameter("wv_loc", [NB, 2, 128, KT * 512], BF, isOutput=False)
    p["wp_loc"] = nc.declare_dram_parameter("wp_loc", [NB, 8, 128, KT * 128], BF, isOutput=False)
    p["wqk_glob"] = nc.declare_dram_parameter("wqk_glob", [NB, 16, 128, KT * 128], BF, isOutput=False)
    p["wv_glob"] = nc.declare_dram_parameter("wv_glob", [NB, 2, 128, KT * 512], BF, isOutput=False)
    p["wp_glob"] = nc.declare_dram_parameter("wp_glob", [NB, 8, 128, KT * 128], BF, isOutput=False)
    p["gT_loc"] = nc.declare_dram_parameter("gT_loc", [NB, 128, KT], F32, isOutput=False)
    p["gT_glob"] = nc.declare_dram_parameter("gT_glob", [NB, 128, KT], F32, isOutput=False)
    p["corner"] = nc.declare_dram_parameter("corner", [128, 128], BF, isOutput=False)
    p["trimem"] = nc.declare_dram_parameter("trimem", [ML, TL], BF, isOutput=False)
    p["mask_memq"] = nc.declare_dram_parameter("mask_memq", [MG, MG], BF, isOutput=False)
    p["onesc"] = nc.declare_dram_parameter("onesc", [128, 1], BF, isOutput=False)
    p["onesr"] = nc.declare_dram_parameter("onesr", [1, 128], BF, isOutput=False)
    p["selp"] = nc.declare_dram_parameter("selp", [97, 4 * 64], BF, isOutput=False)
    p["id128"] = nc.declare_dram_parameter("id128", [128, 128], F32, isOutput=False)
    p["out"] = nc.declare_dram_parameter("out", [2, T, C], F32, isOutput=True)

    with nc.allow_low_precision(reason="bf16 compute, f32 accumulation in PSUM"):
        with tile.TileContext(nc) as tc:
            with ExitStack() as ctx:
                build_body(ctx, tc, nc, p)
    split_excess_waits(nc)
    return nc


MAX_WAITS = 1      # this walrus build: 1 sync wait per instruction


def split_excess_waits(nc):
    """Hoist overflow semaphore waits onto preceding same-engine nops."""
    nsplit = 0
    for f in nc.m.functions:
        for blk in f.blocks:
            il = blk.instructions
            i = 0
            while i < len(il):
                inst = il[i]
                si = getattr(inst, "sync_info", None)
                if si is not None and si.on_wait and len(si.on_wait) > MAX_WAITS:
                    waits = list(si.on_wait)
                    keep = waits[-MAX_WAITS:]
                    excess = waits[:-MAX_WAITS]
                    pos = i
                    while excess:
                        chunk, excess = excess[:MAX_WAITS], excess[MAX_WAITS:]
                        nop = mybir.InstNoOp(name=f"{inst.name}_wsplit{nsplit}", ins=[], outs=[])
                        nsplit += 1
                        nop.engine = inst.engine
                        nop.sync_info = mybir.SyncInfo(on_wait=chunk, on_update=[])
                        nc.register_instruction(nop, overwrite=True)
                        il.insert(pos, nop)
                        pos += 1
                        i += 1
                    inst.sync_info = mybir.SyncInfo(on_wait=keep, on_update=list(si.on_update))
                i += 1
    return nsplit


def build_body(ctx, tc, nc, p):
    const = ctx.enter_context(tc.tile_pool(name="const", bufs=1))
    state = ctx.enter_context(tc.tile_pool(name="state", bufs=1))
    wpool = ctx.enter_context(tc.tile_pool(name="wpool", bufs=1))
    spool = ctx.enter_context(tc.tile_pool(name="spool", bufs=1))
    psum = ctx.enter_context(tc.tile_pool(name="psum", bufs=1, space="PSUM"))
    dram = ctx.enter_context(tc.tile_pool(name="dram", bufs=1, space="DRAM"))

    def pt(shape, name, tag="ps", bufs=4):
        return psum.tile(shape, F32, tag=tag, name=name, bufs=bufs)

    # ---------------- constants ----------------
    corner = const.tile([128, 128], BF, name="corner_sb")
    nc.sync.dma_start(out=corner, in_=p["corner"][:, :])
    trimem = const.tile([ML, TL], BF, name="trimem_sb")
    nc.sync.dma_start(out=trimem, in_=p["trimem"][:, :])
    mask_memq = const.tile([MG, MG], BF, name="mask_memq_sb")
    nc.sync.dma_start(out=mask_memq, in_=p["mask_memq"][:, :])
    onesc = const.tile([128, 1], BF, name="onesc_sb")
    nc.sync.dma_start(out=onesc, in_=p["onesc"][:, :])
    onesr = const.tile([1, 128], BF, name="onesr_sb")
    nc.sync.dma_start(out=onesr, in_=p["onesr"][:, :])
    selp = const.tile([97, 4 * 64], BF, name="selp_sb")
    nc.sync.dma_start(out=selp, in_=p["selp"][:, :])
    id128 = const.tile([128, 128], F32, name="id128_sb")
    nc.sync.dma_start(out=id128, in_=p["id128"][:, :])
    id128b = const.tile([128, 128], BF, name="id128b_sb")
    nc.vector.tensor_copy(id128b, id128)
    zb = const.tile([128, 1], F32, name="zb_sb")
    nc.vector.memset(zb, 0.0)
    zrow = const.tile([1, 128], BF, name="zrow_sb")
    nc.vector.memset(zrow, 0.0)
    ones512 = const.tile([1, 512], BF, name="ones512_sb")
    nc.vector.memset(ones512, 1.0)
    epsb = const.tile([1, 1], F32, name="epsb_sb")
    nc.vector.memset(epsb, EPS)
    gT = {}
    for wt in ("loc", "glob"):
        for i in range(NB):
            g = const.tile([128, KT], F32, name=f"gT_{wt}{i}_sb")
            nc.sync.dma_start(out=g, in_=p[f"gT_{wt}"][i])
            gT[(wt, i)] = g

    # ---------------- persistent state ----------------
    xt = [[state.tile([128, TL], BF, name=f"xt_{s}_{kt}") for kt in range(KT)] for s in range(2)]
    mt = [state.tile([128, 2 * MG], BF, name=f"mt_{kt}") for kt in range(KT)]

    # q/k/y tiles: [128, 544] with both sequences side by side (cols s*TL..)
    qT = [state.tile([128, S2], BF, name=f"qT_{kt}") for kt in range(KT)]
    kT_ = [state.tile([128, S2], BF, name=f"kT_{kt}") for kt in range(KT)]
    yT = [state.tile([128, S2], BF, name=f"yT_{kt}") for kt in range(KT)]
    qTg = [state.tile([128, S2], BF, name=f"qTg_{kt}") for kt in range(KT)]
    kTg = [state.tile([128, S2], BF, name=f"kTg_{kt}") for kt in range(KT)]
    yTg = [state.tile([128, S2], BF, name=f"yTg_{kt}") for kt in range(KT)]
    xg = [[state.tile([128, TL], BF, name=f"xg_{g}_{kt}") for kt in range(KT)] for g in range(2)]

    # v tiles (token-major, NH heads x VW cols, ones column preset)
    def make_v(nm):
        vm = [state.tile([ML, NH * VW], BF, name=f"vm{nm}_{s}") for s in range(2)]
        vx = [[state.tile([128, NH * VW], BF, name=f"vx{nm}_{s}_{j}") for j in range(2)] for s in range(2)]
        for s in range(2):
            nc.vector.memset(vm[s].rearrange("p (h w) -> p h w", h=NH)[:, :, 64:65], 1.0)
            for j in range(2):
                nc.vector.memset(vx[s][j].rearrange("p (h w) -> p h w", h=NH)[:, :, 64:65], 1.0)
        return vm, vx

    vml, vxl = make_v("l")
    vmg, vxg = make_v("g")

    # gathered / exchanged global-attention tiles
    k_all = state.tile([128, 8 * S2], BF, name="k_all")       # per src: [128, 544]
    q_all = [state.tile([128, NQ], BF, name=f"q_all_{b}") for b in range(2)]
    y_all = [state.tile([128, NQ], BF, name=f"y_all_{b}") for b in range(2)]
    vb = [state.tile([128, 8 * 2 * 130], BF, name=f"vb_{b}") for b in range(2)]
    vg_m = [state.tile([MG, 130], BF, name=f"vg_m_{b}") for b in range(2)]

    # ---------------- initial load + transpose ----------------
    with nc.named_scope("init"):
        for s in range(2):
            xtm = [spool.tile([128, C], F32, name=f"xtm_{s}_{rb}", tag=f"xtm_{rb}") for rb in range(2)]
            for rb in range(2):
                nc.sync.dma_start(out=xtm[rb], in_=p["xx"][s, rb * 128:(rb + 1) * 128, :])
            lmm = spool.tile([ML, C], F32, name=f"lmm_{s}", tag="lmm")
            nc.sync.dma_start(out=lmm, in_=p["lm"][s])
            for kt in range(KT):
                for rb in range(2):
                    ps_t = pt([128, 128], f"ps_tr_{s}_{kt}_{rb}")
                    nc.tensor.transpose(ps_t, xtm[rb][:, kt * 128:(kt + 1) * 128], id128)
                    nc.vector.tensor_copy(xt[s][kt][:, ML + rb * 128: ML + (rb + 1) * 128], ps_t)
                ps_t = pt([128, ML], f"ps_trl_{s}_{kt}")
                nc.tensor.transpose(ps_t, lmm[:, kt * 128:(kt + 1) * 128], id128[:ML, :ML])
                nc.vector.tensor_copy(xt[s][kt][:, 0:ML], ps_t)
        for b in range(2):
            memm = spool.tile([MG, C], F32, name=f"memm_{b}", tag="lmm")
            nc.sync.dma_start(out=memm, in_=p["mem"][b])
            for kt in range(KT):
                ps_t = pt([128, MG], f"ps_trm_{b}_{kt}")
                nc.tensor.transpose(ps_t, memm[:, kt * 128:(kt + 1) * 128], id128[:MG, :MG])
                nc.vector.tensor_copy(mt[kt][:, b * MG:(b + 1) * MG], ps_t)

    # ---------------- stage building blocks ----------------
    def qk_norm(i, wname, gv, xs, qdst, kdst, stage):
        """q^T/k^T feature-major into [128, 544] tiles, rms-normed * g."""
        for part, dst in (("q", qdst), ("k", kdst)):
            moff = 0 if part == "q" else 8
            sumsq = [None, None]
            for mtile in range(KT):
                w = wpool.tile([128, KT * 128], BF, tag="wqk", bufs=4, name=f"w_{stage}{i}{part}_{mtile}")
                nc.sync.dma_start(out=w, in_=p[wname][i, moff + mtile])
                ps2 = [pt([128, TL], f"ps_{stage}{i}{part}_{mtile}_{s}") for s in range(2)]
                for kt in range(KT):
                    for s in range(2):
                        nc.tensor.matmul(ps2[s], w[:, kt * 128:(kt + 1) * 128], xs[s][kt],
                                         start=(kt == 0), stop=(kt == KT - 1))
                for s in range(2):
                    dsl = dst[mtile][:, s * TL:(s + 1) * TL]
                    nc.vector.tensor_copy(dsl, ps2[s])
                    sq = spool.tile([128, TL], BF, tag="sq", bufs=3, name=f"sq_{stage}{i}{part}_{mtile}_{s}")
                    nc.vector.tensor_mul(sq, dsl, dsl)
                    if sumsq[s] is None:
                        sumsq[s] = pt([1, TL], f"ss_{stage}{i}{part}_{s}", tag="ssacc", bufs=2)
                    nc.tensor.matmul(sumsq[s], onesc, sq, start=(mtile == 0), stop=(mtile == KT - 1))
            for s in range(2):
                rs = spool.tile([1, TL], F32, tag="rs", bufs=2, name=f"rs_{stage}{i}{part}_{s}")
                nc.scalar.activation(rs, sumsq[s], AF.Sqrt, scale=1.0 / C, bias=epsb)
                rrb = spool.tile([1, TL], BF, tag="rrb", bufs=2, name=f"rrb_{stage}{i}{part}_{s}")
                nc.vector.reciprocal(rrb, rs)
                ps_bc = pt([128, TL], f"bc_{stage}{i}{part}_{s}")
                nc.tensor.matmul(ps_bc, onesr, rrb, start=True, stop=True)
                bc = spool.tile([128, TL], BF, tag="bc", bufs=2, name=f"bcs_{stage}{i}{part}_{s}")
                nc.scalar.activation(bc, ps_bc, AF.Copy)
                for mtile in range(KT):
                    dsl = dst[mtile][:, s * TL:(s + 1) * TL]
                    nc.vector.tensor_mul(dsl, dsl, bc)
                    nc.vector.tensor_scalar_mul(dsl, dsl, gv[:, mtile:mtile + 1])

    def v_compute(i, wname, xs, vm, vx, stage):
        """v token-major with per-head ones columns (preset)."""
        for nch in range(2):
            w = wpool.tile([128, KT * 512], BF, tag="wv", bufs=2, name=f"wv_{stage}{i}_{nch}")
            nc.sync.dma_start(out=w, in_=p[wname][i, nch])
            blocks = [(s, vm[s], 0, ML) for s in range(2)]
            blocks += [(s, vx[s][j], ML + j * 128, 128) for s in range(2) for j in range(2)]
            for s, dst, ts_, rows in blocks:
                ps = pt([128, 512], f"psv_{stage}{i}_{nch}_{s}_{ts_}")
                for kt in range(KT):
                    nc.tensor.matmul(ps[:rows], xs[s][kt][:, ts_:ts_ + rows], w[:, kt * 512:(kt + 1) * 512],
                                     start=(kt == 0), stop=(kt == KT - 1))
                dstv = dst[:rows, nch * 8 * VW:(nch + 1) * 8 * VW].rearrange("p (h w) -> p h w", h=8)
                nc.vector.tensor_copy(dstv[:, :, 0:64], ps[:rows].rearrange("p (h w) -> p h w", h=8))

    def y_raw_write(tag, ps_y, n, wdst, dcol, drow):
        """wdst [64, n] = raw ps_y[0:64, :n]; denominator row -> dcol at partition 32*drow."""
        nc.vector.tensor_copy(wdst, ps_y[0:64, :n])
        nc.scalar.activation(dcol[32 * drow:32 * drow + 1, :n], ps_y[64:65, :n], AF.Copy)

    def y_norm_finish(tag, dcol, n, items):
        """items: up to 4 (raw-y dest, base-partition) pairs; denominators at
        partitions 0/32/64/96 of dcol [97, n].  One reciprocal; per dest:
        selector-broadcast into the dest's partition range + in-place mul."""
        rrb = spool.tile([97, 512], BF, tag="rrbA", bufs=2, name=f"rrbA_{tag}")
        nc.vector.reciprocal(rrb[:, :n], dcol[:, :n])
        for r, (wdst, off) in enumerate(items):
            ps_bc = pt([128, 512], f"bca_{tag}_{r}")
            nc.tensor.matmul(ps_bc[off:off + 64, :n], selp[:, r * 64:(r + 1) * 64], rrb[:, :n],
                             start=True, stop=True)
            bc = spool.tile([128, 512], BF, tag="bcA", bufs=4, name=f"bcA_{tag}_{r}")
            nc.vector.tensor_copy(bc[off:off + 64, :n], ps_bc[off:off + 64, :n])
            nc.vector.tensor_mul(wdst, wdst, bc[off:off + 64, :n])

    def local_attn(i):
        pend = []
        for s in range(2):
            dloc = None
            wdsts = []
            for t in range(KT):
                for hh in range(2):
                    off = hh * 64
                    tag = f"L{i}_{s}_{t}_{hh}"
                    h = 2 * t + hh
                    q = qT[t][off:off + 64, s * TL:(s + 1) * TL]
                    ps_y = pt([VW, TL], f"ya_{tag}", tag="psy", bufs=2)
                    # mem keys (16): visible to everything; triangular vs mem queries
                    ps_s = pt([128, 512], f"sa_{tag}_m")
                    nc.tensor.matmul(ps_s[:ML, :TL], kT_[t][off:off + 64, s * TL:s * TL + ML], q,
                                     start=True, stop=True)
                    es = spool.tile([128, 512], BF, tag="es", bufs=6, name=f"es_{tag}_m")
                    nc.scalar.activation(es[:ML, :TL], ps_s[:ML, :TL], AF.Exp, scale=0.125, bias=zb[:ML])
                    nc.vector.tensor_mul(es[:ML, :TL], es[:ML, :TL], trimem)
                    nc.tensor.matmul(ps_y, vml[s][0:ML, h * VW:(h + 1) * VW], es[:ML, :TL],
                                     start=True, stop=False)
                    # x key block 0 (keys 16:144): queries 16:272
                    ps_s = pt([128, 512], f"sa_{tag}_0")
                    nc.tensor.matmul(ps_s[:, :T], kT_[t][off:off + 64, s * TL + ML:s * TL + ML + 128],
                                     q[:, ML:TL], start=True, stop=True)
                    es = spool.tile([128, 512], BF, tag="es", bufs=6, name=f"es_{tag}_0")
                    nc.scalar.activation(es[:, :T], ps_s[:, :T], AF.Exp, scale=0.125, bias=zb)
                    nc.vector.tensor_mul(es[:, 0:128], es[:, 0:128], corner)
                    nc.tensor.matmul(ps_y[:, ML:TL], vxl[s][0][:, h * VW:(h + 1) * VW], es[:, :T],
                                     start=False, stop=False)
                    # x key block 1 (keys 144:272): queries 144:272
                    ps_s = pt([128, 512], f"sa_{tag}_1")
                    nc.tensor.matmul(ps_s[:, :128], kT_[t][off:off + 64, s * TL + ML + 128:(s + 1) * TL],
                                     q[:, ML + 128:TL], start=True, stop=True)
                    es = spool.tile([128, 512], BF, tag="es", bufs=6, name=f"es_{tag}_1")
                    nc.scalar.activation(es[:, :128], ps_s[:, :128], AF.Exp, scale=0.125, bias=zb)
                    nc.vector.tensor_mul(es[:, :128], es[:, :128], corner)
                    nc.tensor.matmul(ps_y[:, ML + 128:TL], vxl[s][1][:, h * VW:(h + 1) * VW], es[:, :128],
                                     start=False, stop=True)
                    if dloc is None:
                        dloc = spool.tile([97, TL], F32, tag="dloc", bufs=8, name=f"dloc_{tag}")
                        nc.gpsimd.memset(dloc, 1.0)
                    wdst = yT[t][off:off + 64, s * TL:(s + 1) * TL]
                    wdsts.append((wdst, off))
                    y_raw_write(tag, ps_y, TL, wdst, dloc, len(wdsts) - 1)
                    if len(wdsts) == 4:
                        pend.append((f"L{i}_{s}_{t}{hh}", dloc, TL, wdsts))
                        dloc, wdsts = None, []

        for args in pend:
            y_norm_finish(*args)

    def memq(i):
        """global mem-token queries: keys = own-batch mem tokens, causal; all heads."""
        dmem = None
        wdsts = []
        pend = []
        for g in range(2):
            for t in range(KT):
                for hh in range(2):
                    off = hh * 64
                    tag = f"Gm{i}_{g}_{t}_{hh}"
                    h = 2 * t + hh
                    ps_s = pt([MG, MG], f"ms_{tag}", tag="ps", bufs=4)
                    nc.tensor.matmul(ps_s, kTg[t][off:off + 64, g * TL:g * TL + MG],
                                     qTg[t][off:off + 64, g * TL:g * TL + MG], start=True, stop=True)
                    es = spool.tile([MG, MG], BF, tag="esm", bufs=2, name=f"esm_{tag}")
                    nc.scalar.activation(es, ps_s, AF.Exp, scale=0.125, bias=zb[:MG])
                    nc.vector.tensor_mul(es, es, mask_memq)
                    ps_y = pt([VW, MG], f"ym_{tag}", tag="psy", bufs=2)
                    nc.tensor.matmul(ps_y, vmg[g][0:MG, h * VW:(h + 1) * VW], es, start=True, stop=True)
                    if dmem is None:
                        dmem = spool.tile([97, MG], F32, tag="dmem", bufs=8, name=f"dmem_{tag}")
                        nc.gpsimd.memset(dmem, 1.0)
                    wdst = yTg[t][off:off + 64, g * TL:g * TL + MG]
                    wdsts.append((wdst, off))
                    y_raw_write(tag, ps_y, MG, wdst, dmem, len(wdsts) - 1)
                    if len(wdsts) == 4:
                        pend.append((f"Gm{i}_{g}_{t}{hh}", dmem, MG, wdsts))
                        dmem, wdsts = None, []

        for args in pend:
            y_norm_finish(*args)

    def global_attn(i):
        """head-sharded causal attention over gathered q/k/v; writes y_all."""
        pend = []
        for b in range(2):
            dglb = None
            wdsts = []
            for lh in range(2):
                off = lh * 64
                for qc in range(4):
                    tag = f"G{i}_{b}_{lh}_{qc}"
                    qsl = q_all[b][off:off + 64, qc * 512:(qc + 1) * 512]
                    ps_y = pt([VW, 512], f"ya_{tag}", tag="psy", bufs=2)
                    # global mem keys: visible to all x queries (from src 0's copy)
                    ps_s = pt([128, 512], f"sa_{tag}_m")
                    nc.tensor.matmul(ps_s[:MG, :], k_all[off:off + 64, b * TL:b * TL + MG], qsl,
                                     start=True, stop=True)
                    es = spool.tile([128, 512], BF, tag="es", bufs=6, name=f"es_{tag}_m")
                    nc.scalar.activation(es[:MG, :], ps_s[:MG, :], AF.Exp, scale=0.125, bias=zb[:MG])
                    nc.tensor.matmul(ps_y, vg_m[b][0:MG, lh * VW:(lh + 1) * VW], es[:MG, :],
                                     start=True, stop=False)
                    # full + band x key blocks
                    for m in range(4 * qc + 4):
                        jb = (m >> 1) if b == 0 else 7 - (m >> 1)
                        r = m - 4 * qc  # >= 0 for band blocks
                        ks = k_all[off:off + 64, jb * S2 + b * TL + ML + (m & 1) * 128:
                                   jb * S2 + b * TL + ML + (m & 1) * 128 + 128]
                        vs = vb[b][:, (2 * jb + (m & 1)) * 130 + lh * VW:
                                   (2 * jb + (m & 1)) * 130 + (lh + 1) * VW]
                        ps_s = pt([128, 512], f"sa_{tag}_{m}")
                        es = spool.tile([128, 512], BF, tag="es", bufs=6, name=f"es_{tag}_{m}")
                        if r < 0:
                            nc.tensor.matmul(ps_s, ks, qsl, start=True, stop=True)
                            nc.scalar.activation(es, ps_s, AF.Exp, scale=0.125, bias=zb)
                            nc.tensor.matmul(ps_y, vs, es, start=False, stop=False)
                        else:
                            n = 512 - r * 128
                            nc.tensor.matmul(ps_s[:, :n], ks, qsl[:, r * 128:], start=True, stop=True)
                            nc.scalar.activation(es[:, :n], ps_s[:, :n], AF.Exp, scale=0.125, bias=zb)
                            nc.vector.tensor_mul(es[:, :128], es[:, :128], corner)
                            nc.tensor.matmul(ps_y[:, r * 128:], vs, es[:, :n],
                                             start=False, stop=(m == 4 * qc + 3))
                    if dglb is None:
                        dglb = spool.tile([97, 512], F32, tag="dglb", bufs=4, name=f"dglb_{tag}")
                        nc.gpsimd.memset(dglb, 1.0)
                    wdst = y_all[b][off:off + 64, qc * 512:(qc + 1) * 512]
                    wdsts.append((wdst, off))
                    y_raw_write(tag, ps_y, 512, wdst, dglb, len(wdsts) - 1)
                    if len(wdsts) == 4:
                        pend.append((f"G{i}_{b}_{lh}{qc}", dglb, 512, wdsts))
                        dglb, wdsts = None, []
        return pend

    def proj(i, wname, ysrc, wr, stage):
        for mo in range(KT):
            w = wpool.tile([128, KT * 128], BF, tag="wp", bufs=3, name=f"wp_{stage}{i}_{mo}")
            nc.sync.dma_start(out=w, in_=p[wname][i, mo])
            ps2 = [pt([128, TL], f"psp_{stage}{i}_{mo}_{s}") for s in range(2)]
            for kt in range(KT):
                for s in range(2):
                    nc.tensor.matmul(ps2[s], w[:, kt * 128:(kt + 1) * 128], ysrc[kt][:, s * TL:(s + 1) * TL],
                                     start=(kt == 0), stop=(kt == KT - 1))
            for s in range(2):
                wr(s, mo, ps2[s])

    # ================= main blocks =================
    for i in range(NB):
        # ---------- local stage ----------
        with nc.named_scope(f"Lqkv{i}"):
            v_compute(i, "wv_loc", xt, vml, vxl, "L")
            qk_norm(i, "wqk_loc", gT[("loc", i)], xt, qT, kT_, "L")
        with nc.named_scope(f"Lattn{i}"):
            local_attn(i)

        def wr_loc(s, mo, ps, i=i):
            nc.scalar.activation(xt[s][mo], ps, AF.Copy)

        with nc.named_scope(f"Lproj{i}"):
            proj(i, "wp_loc", yT, wr_loc, "L")

        # ---------- global stage ----------
        with nc.named_scope(f"Gqkv{i}"):
            for g in range(2):
                for kt in range(KT):
                    nc.gpsimd.tensor_copy(xg[g][kt][:, 0:MG], mt[kt][:, g * MG:(g + 1) * MG])
                    nc.gpsimd.tensor_copy(xg[g][kt][:, MG:TL], xt[g][kt][:, ML:TL])
            qk_norm(i, "wqk_glob", gT[("glob", i)], xg, qTg, kTg, "G")

            # AllToAll 1: q/k head-slices.  chunk d = [q rows 0:128 ; k rows 128:256]
            a2a_qk_in = dram.tile([8 * 256, S2], BF, name=f"a2aqk_in_{i}")
            a2a_qk_out = dram.tile([8 * 256, S2], BF, name=f"a2aqk_out_{i}")
            for d in range(8):
                nc.sync.dma_start(out=a2a_qk_in[d * 256:d * 256 + 128, :], in_=qTg[d])
                nc.sync.dma_start(out=a2a_qk_in[d * 256 + 128:(d + 1) * 256, :], in_=kTg[d])
            rg = [list(range(NCORE))]
            nc.gpsimd.collective_compute("AllToAll", ALU.bypass, replica_groups=rg,
                                         ins=[a2a_qk_in.opt()], outs=[a2a_qk_out.opt()])

            v_compute(i, "wv_glob", xg, vmg, vxg, "G")

            # AllToAll 2: v head-slices, token-major [544 rows/src, 130]
            a2a_v_in = dram.tile([8 * S2, 130], BF, name=f"a2av_in_{i}")
            a2a_v_out = dram.tile([8 * S2, 130], BF, name=f"a2av_out_{i}")
            vin_p = a2a_v_in.rearrange("(d r) w -> r d w", d=8)
            for s in range(2):
                nc.sync.dma_start(out=vin_p[s * TL:s * TL + MG],
                                    in_=vmg[s].rearrange("p (d w) -> p d w", d=8))
                for j in range(2):
                    nc.sync.dma_start(out=vin_p[s * TL + MG + j * 128:s * TL + MG + (j + 1) * 128],
                                        in_=vxg[s][j].rearrange("p (d w) -> p d w", d=8))
            nc.gpsimd.collective_compute("AllToAll", ALU.bypass, replica_groups=rg,
                                         ins=[a2a_v_in.opt()], outs=[a2a_v_out.opt()])

        # overlap the collectives with mem-query attention
        with nc.named_scope(f"Gmemq{i}"):
            memq(i)

        with nc.named_scope(f"Gattn{i}"):
            # load exchanged tensors
            qko_p = a2a_qk_out.rearrange("(j r) w -> r j w", j=8)
            nc.gpsimd.dma_start(out=k_all.rearrange("p (j w) -> p j w", j=8), in_=qko_p[128:256])
            nc.gpsimd.dma_start(out=q_all[0].rearrange("p (j w) -> p j w", j=8),
                                in_=qko_p[0:128, :, MG:TL])
            for j in range(8):
                nc.gpsimd.dma_start(out=q_all[1][:, (7 - j) * 256:(8 - j) * 256],
                                    in_=a2a_qk_out[j * 256:j * 256 + 128, TL + MG:S2])
            vo_p = a2a_v_out.rearrange("(j r) w -> r j w", j=8)
            for b in range(2):
                for u in range(2):
                    nc.gpsimd.dma_start(
                        out=vb[b].rearrange("p (j u w) -> p j u w", j=8, u=2)[:, :, u],
                        in_=vo_p[b * TL + MG + u * 128:b * TL + MG + (u + 1) * 128])
                nc.gpsimd.dma_start(out=vg_m[b], in_=a2a_v_out[b * TL:b * TL + MG, :])
            for args in global_attn(i):
                y_norm_finish(*args)

            # AllToAll 3: y back to token owners.  chunk d rows 0:128 =
            # [cols 0:256 = b0 y of dest tokens, cols 256:512 = b1]
            a2a_y_in = dram.tile([8 * 128, 512], BF, name=f"a2ay_in_{i}")
            a2a_y_out = dram.tile([8 * 128, 512], BF, name=f"a2ay_out_{i}")
            yin_p = a2a_y_in.rearrange("(d r) w -> r d w", d=8)
            nc.sync.dma_start(out=yin_p[:, :, 0:256], in_=y_all[0].rearrange("p (d w) -> p d w", d=8))
            for d in range(8):
                nc.sync.dma_start(out=a2a_y_in[d * 128:(d + 1) * 128, 256:512],
                                    in_=y_all[1][:, (7 - d) * 256:(8 - d) * 256])
            nc.gpsimd.collective_compute("AllToAll", ALU.bypass, replica_groups=rg,
                                         ins=[a2a_y_in.opt()], outs=[a2a_y_out.opt()])
            for kt in range(KT):
                # yTg x cols (mem cols 0:16 written by memq)
                nc.gpsimd.dma_start(
                    out=yTg[kt].rearrange("p (s w) -> p s w", s=2)[:, :, MG:TL],
                    in_=a2a_y_out[kt * 128:(kt + 1) * 128, :].rearrange("r (s w) -> r s w", s=2))

        def wr_glob(s, mo, ps, i=i):
            nc.scalar.activation(xt[s][mo][:, ML:TL], ps[:, MG:TL], AF.Copy)
            nc.scalar.activation(mt[mo][:, s * MG:(s + 1) * MG], ps[:, 0:MG], AF.Copy)

        with nc.named_scope(f"Gproj{i}"):
            proj(i, "wp_glob", yTg, wr_glob, "G")

    # ================= output =================
    with nc.named_scope("out"):
        for s in range(2):
            osb = [spool.tile([128, C], F32, name=f"osb_{s}_{rb}", tag=f"xtm_{rb}") for rb in range(2)]
            for kt in range(KT):
                for rb in range(2):
                    ps_t = psum.tile([128, 128], BF, tag="ps", bufs=4, name=f"ps_out_{s}_{kt}_{rb}")
                    nc.tensor.transpose(ps_t, xt[s][kt][:, ML + rb * 128: ML + (rb + 1) * 128], id128b)
                    nc.vector.tensor_copy(osb[rb][:, kt * 128:(kt + 1) * 128], ps_t)
            for rb in range(2):
                nc.sync.dma_start(out=p["out"][s, rb * 128:(rb + 1) * 128, :], in_=osb[rb])


_NC_CACHE = None


def get_nc():
    global _NC_CACHE
    if _NC_CACHE is None:
        _NC_CACHE = build_nc()
    return _NC_CACHE


def make_in_maps(x, mem_tokens, local_mem, Wqkv_loc, Wproj_loc, g_loc, Wqkv_glob, Wproj_glob, g_glob):
    x = np.asarray(x, np.float32)
    mem_tokens = np.asarray(mem_tokens, np.float32)
    local_mem = np.asarray(local_mem, np.float32)
    g_loc = np.asarray(g_loc, np.float32)
    g_glob = np.asarray(g_glob, np.float32)

    import ml_dtypes
    bf16 = ml_dtypes.bfloat16

    def tile_qk(w):
        # (NB, C, 3C) -> qk part as [NB, 16, 128, KT*128]
        arr = np.asarray(w, np.float32).reshape(NB, KT, 128, 24, 128)
        qk = arr[:, :, :, 0:16, :].transpose(0, 3, 2, 1, 4).reshape(NB, 16, 128, KT * 128)
        return np.ascontiguousarray(qk).astype(bf16)

    def tile_v(w):
        arr = np.asarray(w, np.float32).reshape(NB, KT, 128, 24, 128)
        v = arr[:, :, :, 16:24, :].reshape(NB, KT, 128, 2, 4, 128)
        v = v.transpose(0, 3, 2, 1, 4, 5).reshape(NB, 2, 128, KT * 512)
        return np.ascontiguousarray(v).astype(bf16)

    def tile_p(w):
        arr = np.asarray(w, np.float32).reshape(NB, KT, 128, 8, 128)
        pr = arr.transpose(0, 3, 2, 1, 4).reshape(NB, 8, 128, KT * 128)
        return np.ascontiguousarray(pr).astype(bf16)

    gT_loc = np.ascontiguousarray(g_loc.reshape(NB, KT, 128).transpose(0, 2, 1))
    gT_glob = np.ascontiguousarray(g_glob.reshape(NB, KT, 128).transpose(0, 2, 1))

    kk = np.arange(128)
    corner = (kk[None, :] >= kk[:, None]).astype(np.float32)          # [k, q]: q >= k
    qq = np.arange(TL)
    km = np.arange(ML)
    trimem = (qq[None, :] >= km[:, None]).astype(np.float32)
    qm = np.arange(MG)
    mask_memq = (qm[None, :] >= qm[:, None]).astype(np.float32)
    onesc = np.ones((128, 1), np.float32)
    onesr = np.ones((1, 128), np.float32)
    selp = np.zeros((97, 4 * 64), np.float32)
    for r_ in range(4):
        selp[32 * r_, r_ * 64:(r_ + 1) * 64] = 1.0
    selp = selp.astype(bf16)
    id128 = np.eye(128, dtype=np.float32)

    base = dict(
        wqk_loc=tile_qk(Wqkv_loc), wv_loc=tile_v(Wqkv_loc), wp_loc=tile_p(Wproj_loc),
        wqk_glob=tile_qk(Wqkv_glob), wv_glob=tile_v(Wqkv_glob), wp_glob=tile_p(Wproj_glob),
        gT_loc=gT_loc, gT_glob=gT_glob,
        corner=corner.astype(bf16), trimem=trimem.astype(bf16), mask_memq=mask_memq.astype(bf16),
        onesc=onesc.astype(bf16), onesr=onesr.astype(bf16), id128=id128, selp=selp,
        mem=np.ascontiguousarray(mem_tokens),
    )

    in_maps = []
    for c in range(NCORE):
        m = dict(base)
        m["xx"] = np.ascontiguousarray(np.stack([x[0, c], x[1, 7 - c]]))
        m["lm"] = np.ascontiguousarray(np.stack([local_mem[0, c], local_mem[1, 7 - c]]))
        in_maps.append(m)
    return in_maps


def kernel(**inputs):
    nc = get_nc()
    in_maps = make_in_maps(**inputs)
    res = run_bass_kernel_spmd(nc, in_maps, list(range(NCORE)))
    out = np.zeros((2, NCORE, T, C), np.float32)
    for c in range(NCORE):
        o = res.results[c]["out"]
        out[0, c] = o[0]
        out[1, 7 - c] = o[1]
    return out
